# revision 25
# baseline (speedup 1.0000x reference)
"""BitMLPBlock Trainium2 kernel: out = x + fc2(gelu(fc1(actquant(x)))).

BitNet b1.58 forward with fp8e4 DoubleRow matmuls (2x PE throughput vs
fp16). Quantized operands are stored as fp8 e4m3:

- x is act-quantized per token to ~int8 magnitudes and cast to e4m3 in one
  DVE scalar_tensor_tensor op (e4m3 rounding of x*127/amax; rel err vs the
  reference int grid ~1.1e-2 on the final output, inside the 2e-2 gate).
- h (gelu output) is stored UNSCALED as e4m3: fp8 relative error is
  scale-invariant, so the per-token h scale of the reference buys nothing;
  dropping it lets the scalar engine's Gelu emit fp8 directly and the fc2
  dequant collapse to a constant. The whole h path is ONE activation op.
- Both transposes to contraction-major run on the DMA xbar as 2-byte
  transposes of the fp8 data VIEWED as u16 pairs (half the bytes of the
  fp16 scheme). A u16 unit carries h-dims (2u, 2u+1), so the transposed
  tile holds, per partition p of block b, the byte-interleaved pair
  (256b+2p, 256b+2p+1). DoubleRow matmuls consume this directly: the
  stationary AP pairs two BLOCKS at fixed byte parity beta (dim1 stride
  256B, M stride 2B, odd base for beta=1 - validated exact on HW), and the
  weights are host-permuted so w[p, i, beta, n] = w[n, 256*(pair_base+i) +
  2p + beta], matching the pairing.
- All xbar transposes issue from the single sync-engine queue (concurrent
  transposes from two queues corrupt).

Sharding: data-parallel over the batch dim (8 batches -> 8 NeuronCores),
weights replicated. No collectives.

Self-contained: hardcodes shapes B=8, T=8192, D=512, H=2048.
"""
import numpy as np

from concourse import bass, mybir, tile
from concourse.bass_utils import run_bass_kernel_spmd
from concourse.vector_clock import ScopedClock

B, T, D, H = 8, 8192, 512, 2048
N_CORES = 8
P = 128                      # partitions / token tile
N_TILES = T // P             # 64 token tiles per core
F32 = mybir.dt.float32
F16 = mybir.dt.float16
F8 = mybir.dt.float8e4
NP_F8 = mybir.dt.np(F8)
DR = mybir.MatmulPerfMode.DoubleRow


# ---------------------------------------------------------------------------
# Workarounds for this container's walrus build, which supports only ONE sync
# wait command per instruction. Tile's tail drain and its add_semaphores pass
# both emit multi-wait instructions; split the extras onto standalone
# wait/NoOp instructions on the same engine.
# ---------------------------------------------------------------------------
_PATCHED = False


def _patch_tile():
    global _PATCHED
    if _PATCHED:
        return
    _PATCHED = True

    def _drain_and_barrier_split(self, tick_clock, wait_clock):
        nc = self.nc
        probe = nc.sync.nop(nofuse=True)
        wait_clock.add_sem_waits(
            probe.ins, ScopedClock({None: tick_clock.global_clock}))
        si = probe.ins.sync_info
        waits = list(si.on_wait) if si is not None and si.on_wait else []
        sems_by_name = {}
        if self.sems is not None:
            for s in self.sems.allocated().values():
                sems_by_name[s.name] = s
        kept = []
        for w in waits:
            sem = sems_by_name.get(w.ant_name)
            if sem is None or w.wait_mode != "sem-ge-imm" or w.wait_value is None:
                kept.append(w)
                continue
            nc.sync.wait_ge(sem, w.wait_value)
        if si is not None:
            si.on_wait = kept
        nc.sync.drain()
        nc.all_engine_barrier()
        assert self.sems is not None
        popped = nc._tile_sem_poison_stack.pop()
        assert popped is self._sem_poison
        nc.clear_and_free_semaphores(list(self.sems.allocated().values()))
        nc.all_engine_barrier()

    tile.TileContext._drain_and_barrier = _drain_and_barrier_split

    orig_commit = tile.TileContext._commit_instruction

    def _commit_split_waits(self, inst, lazy_reg_writes=True):
        si = getattr(inst, "sync_info", None)
        if (
            si is not None
            and si.on_wait
            and len(si.on_wait) > 1
            and inst.engine != mybir.EngineType.Unassigned
        ):
            waits = list(si.on_wait)
            si.on_wait = [waits[-1]]
            for w in waits[:-1]:
                nop = mybir.InstNoOp(
                    name=self.nc.get_next_instruction_name(),
                    text_hint="split_wait",
                    bass_nofuse=True,
                    engine=inst.engine,
                    sync_info=mybir.SyncInfo(on_wait=[w], on_update=[]),
                )
                self._add_instruction(nop)
        return orig_commit(self, inst, lazy_reg_writes)

    tile.TileContext._commit_instruction = _commit_split_waits


_patch_tile()


def build_nc(w1un: float, w2un: float, with_b1: bool):
    """w1un/w2un: host-folded weight unscale constants."""
    nc = bass.Bass("TRN2", target_bir_lowering=False, num_devices=N_CORES)

    x_ext = nc.declare_dram_parameter("x", [T, D], F32, isOutput=False)
    # w1dr[p, i, beta, n] = w1q[n, 256*i + 2*p + beta]
    w1dr_ext = nc.declare_dram_parameter("w1dr", [P, 2, 2, H], F8, isOutput=False)
    # w2dr[p, bp, i, beta, n] = w2q[n, 256*(2*bp+i) + 2*p + beta]
    w2dr_ext = nc.declare_dram_parameter("w2dr", [P, 4, 2, 2, D], F8, isOutput=False)
    b1_ext = None
    if with_b1:
        b1_ext = nc.declare_dram_parameter("b1bc", [P, H], F32, isOutput=False)
    out_ext = nc.declare_dram_parameter("out", [T, D], F32, isOutput=True)

    mm = nc.tensor.matmul
    Alu = mybir.AluOpType
    Act = mybir.ActivationFunctionType

    with tile.TileContext(nc) as tc:
        with (
            tc.tile_pool(name="const", bufs=1) as cpool,
            tc.tile_pool(name="xin", bufs=8) as xpool,
            tc.tile_pool(name="vec", bufs=12) as vpool,
            tc.tile_pool(name="stage", bufs=6) as spool,
            tc.tile_pool(name="big", bufs=6) as bpool,
            tc.tile_pool(name="outp", bufs=4) as opool,
            tc.tile_pool(name="ps_mm1", bufs=4, space="PSUM") as ps_mm1,
            tc.tile_pool(name="ps_2", bufs=4, space="PSUM") as ps_2,
        ):
            # resident weights
            w1dr_sb = cpool.tile([P, 2, 2, H], F8, tag="w1")
            w2dr_sb = cpool.tile([P, 4, 2, 2, D], F8, tag="w2")
            zeros16 = cpool.tile([P, D], F16, tag="z16")
            nc.vector.memset(zeros16[:, :], 0.0)
            b1_sb = None
            if with_b1:
                b1_sb = cpool.tile([P, H], F32, tag="b1")
                nc.gpsimd.dma_start(out=b1_sb[:, :], in_=b1_ext[:, :])

            def load_x_pair(tp):
                """One x-load DMA per pair, issued 2 pairs ahead of the
                quant chain so the vector queue never waits on a fresh
                load at iteration top."""
                row = tp * 2 * P
                x2 = xpool.tile([P, 2, D], F32, tag="x")
                nc.gpsimd.dma_start(
                    out=x2[:, :, :],
                    in_=x_ext[row:row + 2 * P, :].rearrange(
                        "(j p) n -> p j n", p=P))
                return x2

            def stage_a_pair(tp, x2):
                """Act-quant(->fp8) + u16 xbar transpose for tile pair
                (2tp, 2tp+1). ONE transpose per pair (DMA instruction
                issue is a serialized resource)."""
                xq8 = spool.tile([P, 2, D], F8, tag="xq")
                inv1s = []
                for j in range(2):
                    amax = vpool.tile([P, 1], F32, tag="amax")
                    nc.vector.tensor_reduce(
                        amax[:, :], x2[:, j, :], axis=mybir.AxisListType.X,
                        op=Alu.max, apply_absolute_value=True)
                    # t127 = max(amax, 1e-5)/127 ; s_x = 127/max(amax,1e-5)
                    t127 = vpool.tile([P, 1], F32, tag="t127")
                    nc.vector.tensor_scalar(
                        t127[:, :], amax[:, :], 1e-5, 1.0 / 127.0,
                        op0=Alu.max, op1=Alu.mult)
                    s_x = vpool.tile([P, 1], F32, tag="sx")
                    nc.vector.reciprocal(s_x[:, :], t127[:, :])
                    inv1 = vpool.tile([P, 1], F32, tag="inv1")
                    nc.vector.tensor_scalar_mul(inv1[:, :], t127[:, :], w1un)
                    inv1s.append(inv1)
                    # xq8 = e4m3(x * s_x)
                    nc.vector.scalar_tensor_tensor(
                        xq8[:, j, :], x2[:, j, :], s_x[:, :], zeros16[:, :],
                        op0=Alu.mult, op1=Alu.add)

                # 2-byte xbar transpose of the fp8 pair viewed as u16:
                # block k of xT16 is (tile j = k//2, d-block b = k%2);
                # partition p holds d-pair (512j + 256b + 2p, ... + 1).
                xT16 = spool.tile([P, 4, P], F16, tag="xT")
                nc.scalar.dma_start_transpose(
                    out=xT16[:, :, :], in_=xq8[:, :, :].bitcast(F16))
                xT8v = xT16[:, :, :].bitcast(F8).rearrange(
                    "p a (m t) -> p a m t", t=2)
                return x2, inv1s, xT8v

            loaded = []

            def stage_a(tp):
                x2 = loaded.pop(0)
                return stage_a_pair(tp, x2)

            def stage_b(x_t, inv1, xT8v, j, hq8):
                """fc1 (DoubleRow) -> gelu -> fp8, for tile j of the pair."""
                for c in range(4):
                    ps1 = ps_mm1.tile([P, 512], F32, tag="mm1")
                    for beta in range(2):
                        mm(ps1[:, :], xT8v[:, 2 * j:2 * j + 2, :, beta],
                           w1dr_sb[:, :, beta, c * 512:(c + 1) * 512],
                           start=(beta == 0), stop=(beta == 1),
                           perf_mode=DR)
                    if with_b1:
                        hlin = bpool.tile([P, 512], F32, tag="hlin")
                        nc.vector.scalar_tensor_tensor(
                            hlin[:, :], ps1[:, :], inv1[:, :],
                            b1_sb[:, c * 512:(c + 1) * 512],
                            op0=Alu.mult, op1=Alu.add)
                        nc.scalar.activation(
                            hq8[:, j, c * 512:(c + 1) * 512], hlin[:, :],
                            Act.Gelu, bias=0.0, scale=1.0)
                    else:
                        # ONE wide op for the whole h path: dequant + gelu +
                        # e4m3 cast (h stored unscaled; fp8 err is
                        # scale-invariant so the per-token scale buys nothing)
                        nc.scalar.activation(
                            hq8[:, j, c * 512:(c + 1) * 512], ps1[:, :],
                            Act.Gelu, bias=0.0, scale=inv1[:, :])

            def stage_b2(x_t, hT8v, j):
                """fc2 (DoubleRow) matmuls only; dequant+residual lagged."""
                ps2 = ps_2.tile([P, 512], F32, tag="mm2")
                for bp in range(4):
                    for beta in range(2):
                        mm(ps2[:, :], hT8v[:, 8 * j + 2 * bp:8 * j + 2 * bp + 2, :, beta],
                           w2dr_sb[:, bp, :, beta, :],
                           start=(bp == 0 and beta == 0),
                           stop=(bp == 3 and beta == 1), perf_mode=DR)
                return ps2

            # software pipeline over tile PAIRS, two levels deep:
            # - stage A (load/quant/x-transpose) runs LOOKAHEAD_P pairs ahead
            #   so the sync queue has the next x-transpose in flight before
            #   it blocks on the current pair's hq-gated h-transpose.
            # - fc2 of pair tp-B2_LAG is emitted AFTER fc1 of pair tp: the
            #   PE queue is FIFO, so emitting fc2(tp) right after fc1(tp)
            #   would stall the PE ~5-7us per pair waiting on gelu + the
            #   ~4.5us h-transpose (measured), which also re-throttles HAM.
            #   A lag of 2 pairs (~14us of queued PE work) hides it fully.
            NP = N_TILES // 2
            LOOKAHEAD_P = 1
            LOAD_AHEAD = 2
            B2_LAG = 3
            for tp in range(min(LOOKAHEAD_P + LOAD_AHEAD, NP)):
                loaded.append(load_x_pair(tp))
            pending = []
            for tp in range(min(LOOKAHEAD_P, NP)):
                pending.append((tp, *stage_a(tp)))
            # weight DMAs emitted after the prologue x-loads so tile 0's
            # quant chain wins the HBM race at startup
            nc.scalar.dma_start(out=w1dr_sb[:, :, :, :], in_=w1dr_ext[:, :, :, :])
            nc.scalar.dma_start(out=w2dr_sb[:, :, :, :, :], in_=w2dr_ext[:, :, :, :, :])

            res_pending = []

            def run_b2(state):
                tpc, x2, hT8v = state
                ps2s = [stage_b2(x2[:, j, :], hT8v, j) for j in range(2)]
                res_pending.append((tpc, x2, ps2s))

            def flush_res():
                # The residual stt and the out-store are emitted one
                # iteration AFTER their fc2 matmuls: when the vector/gpsimd
                # queues reach them the fc2 results already exist, so
                # neither queue ever stalls on the PE. (An out-stt emitted
                # inline blocks the vector queue -- and with it the next
                # pairs' x-quant chain -- behind fc2 completion: lockstep.)
                tpc, x2, ps2s = res_pending.pop(0)
                out2 = opool.tile([P, 2, D], F32, tag="out")
                for j in range(2):
                    nc.vector.scalar_tensor_tensor(
                        out2[:, j, :], ps2s[j][:, :], w2un, x2[:, j, :],
                        op0=Alu.mult, op1=Alu.add)
                row = tpc * 2 * P
                nc.gpsimd.dma_start(
                    out=out_ext[row:row + 2 * P, :].rearrange(
                        "(j p) n -> p j n", p=P),
                    in_=out2[:, :, :])

            pending_b2 = []
            for tp in range(NP):
                if tp + LOOKAHEAD_P + LOAD_AHEAD < NP:
                    loaded.append(load_x_pair(tp + LOOKAHEAD_P + LOAD_AHEAD))
                if tp + LOOKAHEAD_P < NP:
                    pending.append(
                        (tp + LOOKAHEAD_P, *stage_a(tp + LOOKAHEAD_P)))
                tpc, x2, inv1s, xT8v = pending.pop(0)
                hq8 = bpool.tile([P, 2, H], F8, tag="hq")
                for j in range(2):
                    stage_b(x2[:, j, :], inv1s[j], xT8v, j, hq8)
                # block k of hT16 is (tile j = k//8, h-block b = k%8);
                # partition p holds h-pair (2048j + 256b + 2p, ... + 1).
                # Exactly ONE h-transpose per pair: any finer split (2- or
                # 4-way, both measured) regresses badly -- sync-queue entry
                # count is the most sensitive parameter in this kernel.
                hT16 = bpool.tile([P, 16, P], F16, tag="hT")
                nc.scalar.dma_start_transpose(
                    out=hT16[:, :, :], in_=hq8[:, :, :].bitcast(F16))
                hT8v = hT16[:, :, :].bitcast(F8).rearrange(
                    "p a (m t) -> p a m t", t=2)
                pending_b2.append((tpc, x2, hT8v))
                if len(pending_b2) > B2_LAG:
                    run_b2(pending_b2.pop(0))
                # residual flush at iteration END: its fc2-dependent waits
                # never block the next pair's quant chain on the vector
                # queue (they are satisfied by the time the queue drains).
                if len(res_pending) > 0:
                    flush_res()
            for st in pending_b2:
                run_b2(st)
                flush_res()
            while res_pending:
                flush_res()

    return nc


def _host_weight_quant(w):
    w = np.asarray(w, np.float32)
    scale = 1.0 / np.float32(max(np.mean(np.abs(w), dtype=np.float32), 1e-5))
    tern = np.clip(np.round(w * scale), -1.0, 1.0).astype(np.float32)
    unscale = np.float32(1.0) / scale  # matches reference's division by scale
    return tern, float(unscale)


LAST_RESULTS = None  # test-harness hook: BassKernelResults of last kernel() run


def kernel(x, w1, b1, w2, b2, _trace=False):
    global LAST_RESULTS

    x = np.asarray(x, np.float32)
    w1_tern, w1_un = _host_weight_quant(w1)
    w2_tern, w2_un = _host_weight_quant(w2)

    # w1dr[p, i, beta, n] = w1q[n, d=256i+2p+beta]:
    # w1_tern is [H, D]; transpose to [D, H], reshape D as (i=2, p=128, b=2)
    w1t = np.ascontiguousarray(w1_tern.T)            # [D, H]
    w1dr = np.ascontiguousarray(
        w1t.reshape(2, P, 2, H).transpose(1, 0, 2, 3)).astype(NP_F8)
    # w2dr[p, bp, i, beta, n] = w2q[n, h=256(2bp+i)+2p+beta]:
    # w2_tern is [D, H]; transpose to [H, D], reshape H as (bp=4, i=2, p, b=2)
    w2t = np.ascontiguousarray(w2_tern.T)            # [H, D]
    w2dr = np.ascontiguousarray(
        w2t.reshape(4, 2, P, 2, D).transpose(2, 0, 1, 3, 4)).astype(NP_F8)

    b1 = np.asarray(b1, np.float32)
    b2 = np.asarray(b2, np.float32)
    with_b1 = bool(np.any(b1))

    nc = build_nc(float(np.float32(w1_un)), float(np.float32(w2_un)), with_b1)

    in_maps = []
    for core in range(N_CORES):
        m = {
            "x": np.ascontiguousarray(x[core]),
            "w1dr": w1dr,
            "w2dr": w2dr,
        }
        if with_b1:
            m["b1bc"] = np.ascontiguousarray(
                np.broadcast_to(b1, (P, H)).astype(np.float32))
        in_maps.append(m)

    res = None
    for attempt in range(3):
        try:
            res = run_bass_kernel_spmd(
                nc, in_maps, core_ids=list(range(N_CORES)), trace=_trace)
            break
        except Exception:
            # transient NRT_EXEC_UNIT_UNRECOVERABLE: reset the PJRT client
            # and retry; the wedge clears with a fresh backend.
            if attempt == 2:
                raise
            import time as _time
            try:
                import jax
                jax.clear_caches()
                jax._src.xla_bridge.backends.cache_clear()  # type: ignore
            except Exception:
                pass
            _time.sleep(5.0)
    LAST_RESULTS = res
    out = np.stack([res.results[c]["out"] for c in range(N_CORES)], axis=0)
    if np.any(b2):
        out = out + b2[None, None, :]
    return out.astype(np.float32)


# revision 26
# speedup vs baseline: 1.2133x; 1.2133x over previous
"""BitMLPBlock Trainium2 kernel: out = x + fc2(gelu(fc1(actquant(x)))).

BitNet b1.58 forward with fp8e4 DoubleRow matmuls (2x PE throughput vs
fp16). Quantized operands are stored as fp8 e4m3:

- x is act-quantized per token to ~int8 magnitudes and cast to e4m3 in one
  DVE scalar_tensor_tensor op (e4m3 rounding of x*127/amax; rel err vs the
  reference int grid ~1.1e-2 on the final output, inside the 2e-2 gate).
- h (gelu output) is stored UNSCALED as e4m3: fp8 relative error is
  scale-invariant, so the per-token h scale of the reference buys nothing;
  dropping it lets the scalar engine's Gelu emit fp8 directly and the fc2
  dequant collapse to a constant. The whole h path is ONE activation op.
- Both transposes to contraction-major run on the DMA xbar as 2-byte
  transposes of the fp8 data VIEWED as u16 pairs (half the bytes of the
  fp16 scheme). A u16 unit carries h-dims (2u, 2u+1), so the transposed
  tile holds, per partition p of block b, the byte-interleaved pair
  (256b+2p, 256b+2p+1). DoubleRow matmuls consume this directly: the
  stationary AP pairs two BLOCKS at fixed byte parity beta (dim1 stride
  256B, M stride 2B, odd base for beta=1 - validated exact on HW), and the
  weights are host-permuted so w[p, i, beta, n] = w[n, 256*(pair_base+i) +
  2p + beta], matching the pairing.
- All xbar transposes issue from the single sync-engine queue (concurrent
  transposes from two queues corrupt).

Sharding: data-parallel over the batch dim (8 batches -> 8 NeuronCores),
weights replicated. No collectives.

Self-contained: hardcodes shapes B=8, T=8192, D=512, H=2048.
"""
import numpy as np

from concourse import bass, mybir, tile
from concourse.bass_utils import run_bass_kernel_spmd
from concourse.vector_clock import ScopedClock

B, T, D, H = 8, 8192, 512, 2048
N_CORES = 8
P = 128                      # partitions / token tile
N_TILES = T // P             # 64 token tiles per core
F32 = mybir.dt.float32
F16 = mybir.dt.float16
F8 = mybir.dt.float8e4
NP_F8 = mybir.dt.np(F8)
DR = mybir.MatmulPerfMode.DoubleRow


# ---------------------------------------------------------------------------
# Workarounds for this container's walrus build, which supports only ONE sync
# wait command per instruction. Tile's tail drain and its add_semaphores pass
# both emit multi-wait instructions; split the extras onto standalone
# wait/NoOp instructions on the same engine.
# ---------------------------------------------------------------------------
_PATCHED = False


def _patch_tile():
    global _PATCHED
    if _PATCHED:
        return
    _PATCHED = True

    def _drain_and_barrier_split(self, tick_clock, wait_clock):
        nc = self.nc
        probe = nc.sync.nop(nofuse=True)
        wait_clock.add_sem_waits(
            probe.ins, ScopedClock({None: tick_clock.global_clock}))
        si = probe.ins.sync_info
        waits = list(si.on_wait) if si is not None and si.on_wait else []
        sems_by_name = {}
        if self.sems is not None:
            for s in self.sems.allocated().values():
                sems_by_name[s.name] = s
        kept = []
        for w in waits:
            sem = sems_by_name.get(w.ant_name)
            if sem is None or w.wait_mode != "sem-ge-imm" or w.wait_value is None:
                kept.append(w)
                continue
            nc.sync.wait_ge(sem, w.wait_value)
        if si is not None:
            si.on_wait = kept
        nc.sync.drain()
        nc.all_engine_barrier()
        assert self.sems is not None
        popped = nc._tile_sem_poison_stack.pop()
        assert popped is self._sem_poison
        nc.clear_and_free_semaphores(list(self.sems.allocated().values()))
        nc.all_engine_barrier()

    tile.TileContext._drain_and_barrier = _drain_and_barrier_split

    orig_commit = tile.TileContext._commit_instruction

    def _commit_split_waits(self, inst, lazy_reg_writes=True):
        si = getattr(inst, "sync_info", None)
        if (
            si is not None
            and si.on_wait
            and len(si.on_wait) > 1
            and inst.engine != mybir.EngineType.Unassigned
        ):
            waits = list(si.on_wait)
            si.on_wait = [waits[-1]]
            for w in waits[:-1]:
                nop = mybir.InstNoOp(
                    name=self.nc.get_next_instruction_name(),
                    text_hint="split_wait",
                    bass_nofuse=True,
                    engine=inst.engine,
                    sync_info=mybir.SyncInfo(on_wait=[w], on_update=[]),
                )
                self._add_instruction(nop)
        return orig_commit(self, inst, lazy_reg_writes)

    tile.TileContext._commit_instruction = _commit_split_waits


_patch_tile()


def build_nc(w1un: float, w2un: float, with_b1: bool):
    """w1un/w2un: host-folded weight unscale constants."""
    nc = bass.Bass("TRN2", target_bir_lowering=False, num_devices=N_CORES)

    x_ext = nc.declare_dram_parameter("x", [T, D], F32, isOutput=False)
    # w1dr[p, i, beta, n] = w1q[n, 256*i + 2*p + beta]
    w1dr_ext = nc.declare_dram_parameter("w1dr", [P, 2, 2, H], F8, isOutput=False)
    # w2dr[p, bp, i, beta, n] = w2q[n, 256*(2*bp+i) + 2*p + beta]
    w2dr_ext = nc.declare_dram_parameter("w2dr", [P, 4, 2, 2, D], F8, isOutput=False)
    b1_ext = None
    if with_b1:
        b1_ext = nc.declare_dram_parameter("b1bc", [P, H], F32, isOutput=False)
    out_ext = nc.declare_dram_parameter("out", [T, D], F32, isOutput=True)

    mm = nc.tensor.matmul
    Alu = mybir.AluOpType
    Act = mybir.ActivationFunctionType

    with tile.TileContext(nc) as tc:
        with (
            tc.tile_pool(name="const", bufs=1) as cpool,
            tc.tile_pool(name="xin", bufs=8) as xpool,
            tc.tile_pool(name="vec", bufs=16) as vpool,
            tc.tile_pool(name="stage", bufs=6) as spool,
            tc.tile_pool(name="big", bufs=6) as bpool,
            tc.tile_pool(name="outp", bufs=6) as opool,
            tc.tile_pool(name="ps_mm1", bufs=4, space="PSUM") as ps_mm1,
            tc.tile_pool(name="ps_2", bufs=4, space="PSUM") as ps_2,
        ):
            # resident weights
            w1dr_sb = cpool.tile([P, 2, 2, H], F8, tag="w1")
            w2dr_sb = cpool.tile([P, 4, 2, 2, D], F8, tag="w2")
            zeros16 = cpool.tile([P, D], F16, tag="z16")
            nc.vector.memset(zeros16[:, :], 0.0)
            b1_sb = None
            if with_b1:
                b1_sb = cpool.tile([P, H], F32, tag="b1")
                nc.gpsimd.dma_start(out=b1_sb[:, :], in_=b1_ext[:, :])

            def load_x_pair(tp):
                """One x-load DMA per pair, issued 2 pairs ahead of the
                quant chain so the vector queue never waits on a fresh
                load at iteration top."""
                row = tp * 2 * P
                x2 = xpool.tile([P, 2, D], F32, tag="x")
                nc.gpsimd.dma_start(
                    out=x2[:, :, :],
                    in_=x_ext[row:row + 2 * P, :].rearrange(
                        "(j p) n -> p j n", p=P))
                return x2

            def stage_a_pair(tp, x2):
                """Act-quant(->fp8) + u16 xbar transpose for tile pair
                (2tp, 2tp+1). ONE transpose per pair (DMA instruction
                issue is a serialized resource)."""
                xq8 = spool.tile([P, 2, D], F8, tag="xq")
                inv1s = []
                for j in range(2):
                    amax = vpool.tile([P, 1], F32, tag="amax")
                    nc.vector.tensor_reduce(
                        amax[:, :], x2[:, j, :], axis=mybir.AxisListType.X,
                        op=Alu.max, apply_absolute_value=True)
                    # t127 = max(amax, 1e-5)/127 ; s_x = 127/max(amax,1e-5)
                    t127 = vpool.tile([P, 1], F32, tag="t127")
                    nc.vector.tensor_scalar(
                        t127[:, :], amax[:, :], 1e-5, 1.0 / 127.0,
                        op0=Alu.max, op1=Alu.mult)
                    s_x = vpool.tile([P, 1], F32, tag="sx")
                    nc.vector.reciprocal(s_x[:, :], t127[:, :])
                    inv1 = vpool.tile([P, 1], F32, tag="inv1")
                    nc.vector.tensor_scalar_mul(inv1[:, :], t127[:, :], w1un)
                    inv1s.append(inv1)
                    # xq8 = e4m3(x * s_x)
                    nc.vector.scalar_tensor_tensor(
                        xq8[:, j, :], x2[:, j, :], s_x[:, :], zeros16[:, :],
                        op0=Alu.mult, op1=Alu.add)

                # 2-byte xbar transpose of the fp8 pair viewed as u16:
                # block k of xT16 is (tile j = k//2, d-block b = k%2);
                # partition p holds d-pair (512j + 256b + 2p, ... + 1).
                xT16 = spool.tile([P, 4, P], F16, tag="xT")
                nc.sync.dma_start_transpose(
                    out=xT16[:, :, :], in_=xq8[:, :, :].bitcast(F16))
                xT8v = xT16[:, :, :].bitcast(F8).rearrange(
                    "p a (m t) -> p a m t", t=2)
                return x2, inv1s, xT8v

            loaded = []

            def stage_a(tp):
                x2 = loaded.pop(0)
                return stage_a_pair(tp, x2)

            def stage_b(x_t, inv1, xT8v, j, hq8):
                """fc1 (DoubleRow) -> gelu -> fp8, for tile j of the pair."""
                for c in range(4):
                    ps1 = ps_mm1.tile([P, 512], F32, tag="mm1")
                    for beta in range(2):
                        mm(ps1[:, :], xT8v[:, 2 * j:2 * j + 2, :, beta],
                           w1dr_sb[:, :, beta, c * 512:(c + 1) * 512],
                           start=(beta == 0), stop=(beta == 1),
                           perf_mode=DR)
                    if with_b1:
                        hlin = bpool.tile([P, 512], F32, tag="hlin")
                        nc.vector.scalar_tensor_tensor(
                            hlin[:, :], ps1[:, :], inv1[:, :],
                            b1_sb[:, c * 512:(c + 1) * 512],
                            op0=Alu.mult, op1=Alu.add)
                        nc.scalar.activation(
                            hq8[:, j, c * 512:(c + 1) * 512], hlin[:, :],
                            Act.Gelu, bias=0.0, scale=1.0)
                    else:
                        # ONE wide op for the whole h path: dequant + gelu +
                        # e4m3 cast (h stored unscaled; fp8 err is
                        # scale-invariant so the per-token scale buys nothing)
                        nc.scalar.activation(
                            hq8[:, j, c * 512:(c + 1) * 512], ps1[:, :],
                            Act.Gelu, bias=0.0, scale=inv1[:, :])

            def stage_b2(x_t, hT8v, j):
                """fc2 (DoubleRow) matmuls only; dequant+residual lagged."""
                ps2 = ps_2.tile([P, 512], F32, tag="mm2")
                for bp in range(4):
                    for beta in range(2):
                        mm(ps2[:, :], hT8v[:, 8 * j + 2 * bp:8 * j + 2 * bp + 2, :, beta],
                           w2dr_sb[:, bp, :, beta, :],
                           start=(bp == 0 and beta == 0),
                           stop=(bp == 3 and beta == 1), perf_mode=DR)
                return ps2

            # software pipeline over tile PAIRS, two levels deep:
            # - stage A (load/quant/x-transpose) runs LOOKAHEAD_P pairs ahead
            #   so the sync queue has the next x-transpose in flight before
            #   it blocks on the current pair's hq-gated h-transpose.
            # - fc2 of pair tp-B2_LAG is emitted AFTER fc1 of pair tp: the
            #   PE queue is FIFO, so emitting fc2(tp) right after fc1(tp)
            #   would stall the PE ~5-7us per pair waiting on gelu + the
            #   ~4.5us h-transpose (measured), which also re-throttles HAM.
            #   A lag of 2 pairs (~14us of queued PE work) hides it fully.
            NP = N_TILES // 2
            LOOKAHEAD_P = 1
            LOAD_AHEAD = 2
            B2_LAG = 3
            for tp in range(min(LOOKAHEAD_P + LOAD_AHEAD, NP)):
                loaded.append(load_x_pair(tp))
            pending = []
            for tp in range(min(LOOKAHEAD_P, NP)):
                pending.append((tp, *stage_a(tp)))
            # weight DMAs emitted after the prologue x-loads so tile 0's
            # quant chain wins the HBM race at startup
            nc.scalar.dma_start(out=w1dr_sb[:, :, :, :], in_=w1dr_ext[:, :, :, :])
            nc.scalar.dma_start(out=w2dr_sb[:, :, :, :, :], in_=w2dr_ext[:, :, :, :, :])

            res_pending = []

            def run_b2(state):
                tpc, x2, hT8v = state
                ps2s = [stage_b2(x2[:, j, :], hT8v, j) for j in range(2)]
                res_pending.append((tpc, x2, ps2s))

            def flush_res():
                # The residual stt and the out-store are emitted one
                # iteration AFTER their fc2 matmuls: when the vector/gpsimd
                # queues reach them the fc2 results already exist, so
                # neither queue ever stalls on the PE. (An out-stt emitted
                # inline blocks the vector queue -- and with it the next
                # pairs' x-quant chain -- behind fc2 completion: lockstep.)
                tpc, x2, ps2s = res_pending.pop(0)
                out2 = opool.tile([P, 2, D], F32, tag="out")
                for j in range(2):
                    nc.vector.scalar_tensor_tensor(
                        out2[:, j, :], ps2s[j][:, :], w2un, x2[:, j, :],
                        op0=Alu.mult, op1=Alu.add)
                row = tpc * 2 * P
                nc.gpsimd.dma_start(
                    out=out_ext[row:row + 2 * P, :].rearrange(
                        "(j p) n -> p j n", p=P),
                    in_=out2[:, :, :])

            pending_b2 = []
            for tp in range(NP):
                if tp + LOOKAHEAD_P + LOAD_AHEAD < NP:
                    loaded.append(load_x_pair(tp + LOOKAHEAD_P + LOAD_AHEAD))
                if tp + LOOKAHEAD_P < NP:
                    pending.append(
                        (tp + LOOKAHEAD_P, *stage_a(tp + LOOKAHEAD_P)))
                tpc, x2, inv1s, xT8v = pending.pop(0)
                hq8 = bpool.tile([P, 2, H], F8, tag="hq")
                for j in range(2):
                    stage_b(x2[:, j, :], inv1s[j], xT8v, j, hq8)
                # block k of hT16 is (tile j = k//8, h-block b = k%8);
                # partition p holds h-pair (2048j + 256b + 2p, ... + 1).
                # Exactly ONE h-transpose per pair: any finer split (2- or
                # 4-way, both measured) regresses badly -- sync-queue entry
                # count is the most sensitive parameter in this kernel.
                hT16 = bpool.tile([P, 16, P], F16, tag="hT")
                nc.sync.dma_start_transpose(
                    out=hT16[:, :, :], in_=hq8[:, :, :].bitcast(F16))
                hT8v = hT16[:, :, :].bitcast(F8).rearrange(
                    "p a (m t) -> p a m t", t=2)
                pending_b2.append((tpc, x2, hT8v))
                if len(pending_b2) > B2_LAG:
                    run_b2(pending_b2.pop(0))
                # residual flush at iteration END: its fc2-dependent waits
                # never block the next pair's quant chain on the vector
                # queue (they are satisfied by the time the queue drains).
                if len(res_pending) > 0:
                    flush_res()
            for st in pending_b2:
                run_b2(st)
                flush_res()
            while res_pending:
                flush_res()

    return nc


def _host_weight_quant(w):
    w = np.asarray(w, np.float32)
    scale = 1.0 / np.float32(max(np.mean(np.abs(w), dtype=np.float32), 1e-5))
    tern = np.clip(np.round(w * scale), -1.0, 1.0).astype(np.float32)
    unscale = np.float32(1.0) / scale  # matches reference's division by scale
    return tern, float(unscale)


LAST_RESULTS = None  # test-harness hook: BassKernelResults of last kernel() run


def kernel(x, w1, b1, w2, b2, _trace=False):
    global LAST_RESULTS

    x = np.asarray(x, np.float32)
    w1_tern, w1_un = _host_weight_quant(w1)
    w2_tern, w2_un = _host_weight_quant(w2)

    # w1dr[p, i, beta, n] = w1q[n, d=256i+2p+beta]:
    # w1_tern is [H, D]; transpose to [D, H], reshape D as (i=2, p=128, b=2)
    w1t = np.ascontiguousarray(w1_tern.T)            # [D, H]
    w1dr = np.ascontiguousarray(
        w1t.reshape(2, P, 2, H).transpose(1, 0, 2, 3)).astype(NP_F8)
    # w2dr[p, bp, i, beta, n] = w2q[n, h=256(2bp+i)+2p+beta]:
    # w2_tern is [D, H]; transpose to [H, D], reshape H as (bp=4, i=2, p, b=2)
    w2t = np.ascontiguousarray(w2_tern.T)            # [H, D]
    w2dr = np.ascontiguousarray(
        w2t.reshape(4, 2, P, 2, D).transpose(2, 0, 1, 3, 4)).astype(NP_F8)

    b1 = np.asarray(b1, np.float32)
    b2 = np.asarray(b2, np.float32)
    with_b1 = bool(np.any(b1))

    nc = build_nc(float(np.float32(w1_un)), float(np.float32(w2_un)), with_b1)

    in_maps = []
    for core in range(N_CORES):
        m = {
            "x": np.ascontiguousarray(x[core]),
            "w1dr": w1dr,
            "w2dr": w2dr,
        }
        if with_b1:
            m["b1bc"] = np.ascontiguousarray(
                np.broadcast_to(b1, (P, H)).astype(np.float32))
        in_maps.append(m)

    res = None
    for attempt in range(3):
        try:
            res = run_bass_kernel_spmd(
                nc, in_maps, core_ids=list(range(N_CORES)), trace=_trace)
            break
        except Exception:
            # transient NRT_EXEC_UNIT_UNRECOVERABLE: reset the PJRT client
            # and retry; the wedge clears with a fresh backend.
            if attempt == 2:
                raise
            import time as _time
            try:
                import jax
                jax.clear_caches()
                jax._src.xla_bridge.backends.cache_clear()  # type: ignore
            except Exception:
                pass
            _time.sleep(5.0)
    LAST_RESULTS = res
    out = np.stack([res.results[c]["out"] for c in range(N_CORES)], axis=0)
    if np.any(b2):
        out = out + b2[None, None, :]
    return out.astype(np.float32)
